# revision 9
# baseline (speedup 1.0000x reference)
"""Causal multi-head attention block, sharded over 8 TRN2 NeuronCores.

Sharding: core c handles batch b = c//2 and head-group g = c%2 (8 of 16 heads).
Each core computes QKV projections, causal flash-style attention, and a
partial output projection for its head group; the host sums the two
head-group partials per batch (partial-sum unshard) and adds b_O.

On-device layouts (per core, S=2048, M=1024, H8=8 heads, Dh=64):
  x_t     [1024, 2048]  x[b] transposed (host pre-transposes)     bf16
  QT/KT   4 pair-tiles [128, 2048]: partition = (head-in-pair, d) bf16
  Vones   16 s-tiles [128, 520]: row=key pos, col=65*h+d, d=64 → 1.0 bf16
  zT_all  4 pair-tiles [128, 2048] (normalized z^T)               bf16
  out_t   [1024, 2048]  partial (out proj)^T, host sums + transposes
Scores are computed transposed (S^T[key, query]) so softmax denominators
come from an extra all-ones column in V (matmul partition reduction), and
no on-chip transposes are needed anywhere.
"""

import sys

if "/opt/trn_rl_repo" not in sys.path:
    sys.path.insert(0, "/opt/trn_rl_repo")

import numpy as np
import ml_dtypes

import concourse.bass as bass
import concourse.mybir as mybir
from concourse import tile

BF16 = mybir.dt.bfloat16
F32 = mybir.dt.float32

B, S, M, H, DH = 4, 2048, 1024, 16, 64
H8 = 8          # heads per core
NP = 4          # head pairs per core
SB = 512        # query superblock
KB = 128        # key block
NSB = S // SB   # 4
NKB = S // KB   # 16
MK = M // 128   # 8 contraction chunks
ATTN_SCALE = 1.0 / np.sqrt(DH)

# ---------------------------------------------------------------------------
# Patch: this walrus build rejects >1 sync-wait per engine instruction.
# Post-pass: for any non-DMA instruction with N>1 waits, insert N-1
# single-wait NoOps on the same engine immediately before it.
MAX_ENGINE_WAITS = 1


def split_multi_waits(nc: bass.Bass):
    n_split = 0
    for f in nc.m.functions:
        for blk in f.blocks:
            new_list = []
            for inst in blk.instructions:
                si = getattr(inst, "sync_info", None)
                waits = list(si.on_wait) if si is not None else []
                if len(waits) > MAX_ENGINE_WAITS:
                    extra = waits[: -MAX_ENGINE_WAITS]
                    keep = waits[-MAX_ENGINE_WAITS:]
                    for i in range(0, len(extra), MAX_ENGINE_WAITS):
                        nop = mybir.InstNoOp(
                            name=f"I-wsplit-{nc.next_id()}", ins=[], outs=[]
                        )
                        nop.engine = inst.engine
                        nop.sync_info = mybir.SyncInfo(
                            on_wait=extra[i : i + MAX_ENGINE_WAITS], on_update=[]
                        )
                        new_list.append(nop)
                    inst.sync_info = mybir.SyncInfo(
                        on_wait=keep, on_update=list(si.on_update)
                    )
                    n_split += 1
                new_list.append(inst)
            blk.instructions = new_list
    return n_split
# ---------------------------------------------------------------------------


def build_nc() -> bass.Bass:
    nc = bass.Bass()

    x_t = nc.declare_dram_parameter("x_t", [M, S], BF16, isOutput=False)
    w_q = nc.declare_dram_parameter("w_q", [M, H8 * DH], BF16, isOutput=False)
    w_k = nc.declare_dram_parameter("w_k", [M, H8 * DH], BF16, isOutput=False)
    w_v = nc.declare_dram_parameter("w_v", [M, H8 * DH], BF16, isOutput=False)
    w_o = nc.declare_dram_parameter("w_o", [H8 * DH, M], BF16, isOutput=False)
    b_q = nc.declare_dram_parameter("b_q", [NP, 128], F32, isOutput=False)
    b_k = nc.declare_dram_parameter("b_k", [NP, 128], F32, isOutput=False)
    b_v = nc.declare_dram_parameter("b_v", [1, H8 * DH], BF16, isOutput=False)
    out_t = nc.declare_dram_parameter("out_t", [M, S], BF16, isOutput=True)

    with tile.TileContext(nc) as tc:
        with (
            tc.tile_pool(name="persist", bufs=1) as persist,
            tc.tile_pool(name="wstream", bufs=1) as wpool,
        ):
            # --- resident tiles -------------------------------------------
            xt = [persist.tile([128, S], BF16, tag=f"xt{k}", name=f"xt{k}") for k in range(MK)]
            qt = [persist.tile([128, S], BF16, tag=f"qt{p}", name=f"qt{p}") for p in range(NP)]
            kt = [persist.tile([128, S], BF16, tag=f"kt{p}", name=f"kt{p}") for p in range(NP)]
            vones = [
                persist.tile([128, H8 * 65], BF16, tag=f"vones{sb}", name=f"vones{sb}")
                for sb in range(NKB)
            ]
            zt = [persist.tile([128, S], BF16, tag=f"zt{p}", name=f"zt{p}") for p in range(NP)]

            wq = [
                [wpool.tile([128, 128], BF16, tag=f"wq{k}_{p}", name=f"wq{k}_{p}") for p in range(NP)]
                for k in range(MK)
            ]
            wk = [
                [wpool.tile([128, 128], BF16, tag=f"wk{k}_{p}", name=f"wk{k}_{p}") for p in range(NP)]
                for k in range(MK)
            ]
            wv = [wpool.tile([128, H8 * DH], BF16, tag=f"wv{k}", name=f"wv{k}") for k in range(MK)]
            wo = [
                [wpool.tile([128, 128], BF16, tag=f"wo{c}_{k}", name=f"wo{c}_{k}") for k in range(MK)]
                for c in range(NP)
            ]
            bq_t = persist.tile([128, NP], F32, tag="bq")
            bk_t = persist.tile([128, NP], F32, tag="bk")
            bv_t = persist.tile([1, H8 * DH], BF16, tag="bv")
            ones_col = persist.tile([1, 128], BF16, tag="ones_col")

            # --- loads ----------------------------------------------------
            for k in range(MK):
                nc.sync.dma_start(xt[k][:], x_t[k * 128 : (k + 1) * 128, :])
            for k in range(MK):
                for p in range(NP):
                    nc.sync.dma_start(
                        wq[k][p][:],
                        w_q[k * 128 : (k + 1) * 128, p * 128 : (p + 1) * 128],
                    )
                    nc.sync.dma_start(
                        wk[k][p][:],
                        w_k[k * 128 : (k + 1) * 128, p * 128 : (p + 1) * 128],
                    )
                nc.sync.dma_start(wv[k][:], w_v[k * 128 : (k + 1) * 128, :])
            for c in range(NP):
                for k in range(MK):
                    nc.sync.dma_start(
                        wo[c][k][:],
                        w_o[c * 128 : (c + 1) * 128, k * 128 : (k + 1) * 128],
                    )
            for p in range(NP):
                nc.sync.dma_start(bq_t[:, p], b_q[p])
                nc.sync.dma_start(bk_t[:, p], b_k[p])
            nc.sync.dma_start(bv_t[:], b_v[:])
            nc.gpsimd.memset(ones_col[:], 1.0)
            for sb in range(NKB):
                v3 = vones[sb][:].rearrange("p (h e) -> p h e", e=65)
                nc.gpsimd.memset(v3[:, :, 64:65], 1.0)

            # --- phase 1: QKV projections ---------------------------------
            with tc.tile_pool(name="qkv_ps", bufs=4, space="PSUM") as qkv_ps:
                for sb in range(NSB):
                    ssl = slice(sb * SB, (sb + 1) * SB)
                    for p in range(NP):
                        ps_q = qkv_ps.tile([128, SB], F32, tag="ps_qkv", name="ps_qkv")
                        for k in range(MK):
                            nc.tensor.matmul(
                                ps_q[:],
                                wq[k][p][:],
                                xt[k][:, ssl],
                                start=(k == 0),
                                stop=(k == MK - 1),
                            )
                        nc.vector.tensor_scalar_add(
                            qt[p][:, ssl], ps_q[:], bq_t[:, p : p + 1]
                        )
                        ps_k = qkv_ps.tile([128, SB], F32, tag="ps_qkv", name="ps_qkv")
                        for k in range(MK):
                            nc.tensor.matmul(
                                ps_k[:],
                                wk[k][p][:],
                                xt[k][:, ssl],
                                start=(k == 0),
                                stop=(k == MK - 1),
                            )
                        nc.vector.tensor_scalar_add(
                            kt[p][:, ssl], ps_k[:], bk_t[:, p : p + 1]
                        )
                    # V for the 4 key-blocks of this superblock (natural layout)
                    for kb4 in range(4):
                        kb = sb * 4 + kb4
                        ksl = slice(kb * KB, (kb + 1) * KB)
                        ps_v = qkv_ps.tile([128, H8 * DH], F32, tag="ps_qkv", name="ps_v")
                        for k in range(MK):
                            nc.tensor.matmul(
                                ps_v[:],
                                xt[k][:, ksl],
                                wv[k][:],
                                start=(k == 0),
                                stop=False,
                            )
                        nc.tensor.matmul(
                            ps_v[:], ones_col[:], bv_t[:], start=False, stop=True
                        )
                        v3 = vones[kb][:].rearrange("p (h e) -> p h e", e=65)
                        nc.vector.tensor_copy(
                            v3[:, :, 0:64],
                            ps_v[:].rearrange("p (h e) -> p h e", e=64),
                        )

            # --- phase 2: attention ---------------------------------------
            with (
                tc.tile_pool(name="s_ps", bufs=3, space="PSUM") as s_ps,
                tc.tile_pool(name="z_ps", bufs=3, space="PSUM") as z_ps,
                tc.tile_pool(name="epool", bufs=6) as epool,
                tc.tile_pool(name="npool", bufs=4) as npool,
            ):
                for p in range(NP):
                    for j in range(NSB):
                        qsl = slice(j * SB, (j + 1) * SB)
                        nk = 4 * (j + 1)
                        zps = [
                            z_ps.tile([65, SB], F32, tag="zps", name="zps0"),
                            z_ps.tile([65, SB], F32, tag="zps", name="zps1"),
                        ]
                        for kb in range(nk):
                            ksl = slice(kb * KB, (kb + 1) * KB)
                            es = []
                            for h2 in range(2):
                                rows = slice(h2 * 64, h2 * 64 + 64)
                                sps = s_ps.tile([128, SB], F32, tag="sps", name="sps")
                                nc.tensor.matmul(
                                    sps[:], kt[p][rows, ksl], qt[p][rows, qsl]
                                )
                                e = epool.tile([128, SB], BF16, tag="e", name="e")
                                nc.scalar.activation(
                                    e[:],
                                    sps[:],
                                    mybir.ActivationFunctionType.Exp,
                                    scale=float(ATTN_SCALE),
                                )
                                if kb >= 4 * j:
                                    # zero out non-causal: keep qc - kr + base >= 0
                                    nc.gpsimd.affine_select(
                                        out=e[:],
                                        in_=e[:],
                                        compare_op=mybir.AluOpType.is_ge,
                                        fill=0.0,
                                        base=j * SB - kb * KB,
                                        pattern=[[1, SB]],
                                        channel_multiplier=-1,
                                    )
                                es.append(e)
                            for h2 in range(2):
                                h = 2 * p + h2
                                nc.tensor.matmul(
                                    zps[h2][:],
                                    vones[kb][:, 65 * h : 65 * h + 65],
                                    es[h2][:],
                                    start=(kb == 0),
                                    stop=(kb == nk - 1),
                                )
                        for h2 in range(2):
                            rows = slice(h2 * 64, h2 * 64 + 64)
                            recip = npool.tile([1, SB], BF16, tag="recip", name="recip")
                            with nc.allow_low_precision(reason="softmax recip to bf16 broadcast"):
                                nc.vector.reciprocal(recip[:], zps[h2][64:65, :])
                            bc_ps = z_ps.tile([64, SB], F32, tag="bc_ps", name="bc_ps", bufs=2)
                            nc.tensor.matmul(bc_ps[:], ones_col[:, 0:64], recip[:])
                            bcast = npool.tile([64, SB], F32, tag="bcast", name="bcast")
                            nc.vector.tensor_copy(bcast[:], bc_ps[:])
                            nc.vector.tensor_mul(
                                zt[p][rows, qsl], zps[h2][0:64, :], bcast[:]
                            )

            # --- phase 3: output projection -------------------------------
            with (
                tc.tile_pool(name="o_ps", bufs=4, space="PSUM") as o_ps,
                tc.tile_pool(name="opool", bufs=4) as opool,
            ):
                for k in range(MK):
                    for j in range(NSB):
                        qsl = slice(j * SB, (j + 1) * SB)
                        ps_o = o_ps.tile([128, SB], F32, tag="ps_o", name="ps_o")
                        for c in range(NP):
                            nc.tensor.matmul(
                                ps_o[:],
                                wo[c][k][:],
                                zt[c][:, qsl],
                                start=(c == 0),
                                stop=(c == NP - 1),
                            )
                        ot = opool.tile([128, SB], BF16, tag="ot", name="ot")
                        nc.vector.tensor_copy(ot[:], ps_o[:])
                        nc.sync.dma_start(
                            out_t[k * 128 : (k + 1) * 128, qsl], ot[:]
                        )

    split_multi_waits(nc)
    return nc


_CACHED = {}


def _get_nc():
    if "nc" not in _CACHED:
        _CACHED["nc"] = build_nc()
    return _CACHED["nc"]


def kernel(
    x,
    pos_embed,
    W_Q,
    b_Q,
    W_K,
    b_K,
    W_V,
    b_V,
    W_O,
    b_O,
    _want_results=False,
    _trace=False,
):
    from concourse.bass_utils import run_bass_kernel_spmd

    bf16 = ml_dtypes.bfloat16
    x = np.asarray(x, np.float32)
    W_Q = np.asarray(W_Q, np.float32)
    b_Q = np.asarray(b_Q, np.float32)
    W_K = np.asarray(W_K, np.float32)
    b_K = np.asarray(b_K, np.float32)
    W_V = np.asarray(W_V, np.float32)
    b_V = np.asarray(b_V, np.float32)
    W_O = np.asarray(W_O, np.float32)
    b_O = np.asarray(b_O, np.float32)

    in_maps = []
    for c in range(8):
        b, g = divmod(c, 2)
        hs = slice(g * H8, (g + 1) * H8)
        # [H8, M, DH] -> [M, H8*DH] with col = 64*h + d (pair-major for Q/K)
        wq = np.ascontiguousarray(W_Q[hs].transpose(1, 0, 2).reshape(M, H8 * DH))
        wk = np.ascontiguousarray(W_K[hs].transpose(1, 0, 2).reshape(M, H8 * DH))
        wv = np.ascontiguousarray(W_V[hs].transpose(1, 0, 2).reshape(M, H8 * DH))
        wo = np.ascontiguousarray(W_O[hs].reshape(H8 * DH, M))
        in_maps.append(
            {
                "x_t": np.ascontiguousarray(x[b].T).astype(bf16),
                "w_q": wq.astype(bf16),
                "w_k": wk.astype(bf16),
                "w_v": wv.astype(bf16),
                "w_o": wo.astype(bf16),
                "b_q": np.ascontiguousarray(b_Q[hs].reshape(NP, 128)),
                "b_k": np.ascontiguousarray(b_K[hs].reshape(NP, 128)),
                "b_v": b_V[hs].reshape(1, H8 * DH).astype(bf16),
            }
        )

    nc = _get_nc()
    res = run_bass_kernel_spmd(nc, in_maps, list(range(8)), trace=_trace)

    out = np.empty((B, S, M), np.float32)
    for b in range(B):
        p0 = res.results[2 * b]["out_t"].astype(np.float32)
        p1 = res.results[2 * b + 1]["out_t"].astype(np.float32)
        out[b] = (p0 + p1).T + b_O
    if _want_results:
        return out, res
    return out


# revision 24
# speedup vs baseline: 1.7592x; 1.7592x over previous
"""Causal multi-head attention block, sharded over 8 TRN2 NeuronCores.

Sharding: core c handles batch b = c//2 and head-group g = c%2 (8 of 16 heads).
Each core computes QKV projections, causal flash-style attention, and a
partial output projection for its head group; the host sums the two
head-group partials per batch (partial-sum unshard) and adds b_O.

On-device layouts (per core, S=2048, M=1024, H8=8 heads, Dh=64):
  x_t     [1024, 2048]  x[b] transposed (host pre-transposes)     bf16
  QT/KT   4 pair-tiles [128, 2048]: partition = (head-in-pair, d) bf16
  Vones   16 s-tiles [128, 520]: row=key pos, col=65*h+d, d=64 → 1.0 bf16
  zT_all  4 pair-tiles [128, 2048] (normalized z^T)               bf16
  out_t   [1024, 2048]  partial (out proj)^T, host sums + transposes
Scores are computed transposed (S^T[key, query]) so softmax denominators
come from an extra all-ones column in V (matmul partition reduction), and
no on-chip transposes are needed anywhere.
"""

import sys

if "/opt/trn_rl_repo" not in sys.path:
    sys.path.insert(0, "/opt/trn_rl_repo")

import numpy as np
import ml_dtypes

import concourse.bass as bass
import concourse.mybir as mybir
from concourse import tile

BF16 = mybir.dt.bfloat16
F32 = mybir.dt.float32

B, S, M, H, DH = 4, 2048, 1024, 16, 64
H8 = 8          # heads per core
NP = 4          # head pairs per core
SB = 512        # query superblock
KB = 128        # key block
NSB = S // SB   # 4
NKB = S // KB   # 16
MK = M // 128   # 8 contraction chunks
ATTN_SCALE = 1.0 / np.sqrt(DH)

# ---------------------------------------------------------------------------
# Patch: this walrus build rejects >1 sync-wait per engine instruction.
# Post-pass: for any non-DMA instruction with N>1 waits, insert N-1
# single-wait NoOps on the same engine immediately before it.
MAX_ENGINE_WAITS = 1


def strip_tile_positions(nc: bass.Bass):
    # walrus's LDW optimization refuses instructions carrying tile_position;
    # bass auto-derives it from AP base partitions, which walrus can also do.
    n = 0
    for f in nc.m.functions:
        for blk in f.blocks:
            for inst in blk.instructions:
                if isinstance(inst, (mybir.InstLdweights, mybir.InstMatmult)):
                    if getattr(inst, "tile_position", None) is not None:
                        inst.tile_position = None
                        n += 1
                    if getattr(inst, "tile_size", None) is not None:
                        inst.tile_size = None
    return n


def refuse_ldweights(nc: bass.Bass):
    # Tile's lowering splits each matmul into standalone Ldweights + Matmult
    # (ldweights=False). Re-fuse: drop the Ldweights, mark the matmult
    # self-loading, and merge the Ldweights' semaphore waits into it
    # (split_multi_waits runs after and redistributes >1-wait cases).
    n = 0
    for f in nc.m.functions:
        for blk in f.blocks:
            new_list = []
            pend_waits = []
            pend_updates = []
            for inst in blk.instructions:
                if isinstance(inst, mybir.InstLdweights):
                    si = inst.sync_info
                    if si is not None:
                        pend_waits.extend(list(si.on_wait))
                        pend_updates.extend(list(si.on_update))
                    n += 1
                    continue
                if isinstance(inst, mybir.InstMatmult) and (
                    pend_waits or pend_updates
                ):
                    inst.ldweights = True
                    si = inst.sync_info
                    w = list(si.on_wait) if si else []
                    u = list(si.on_update) if si else []
                    inst.sync_info = mybir.SyncInfo(
                        on_wait=pend_waits + w, on_update=u + pend_updates
                    )
                    pend_waits, pend_updates = [], []
                elif isinstance(inst, mybir.InstMatmult):
                    inst.ldweights = True
                new_list.append(inst)
            assert not pend_waits and not pend_updates
            blk.instructions = new_list
    return n


def split_multi_waits(nc: bass.Bass):
    n_split = 0
    for f in nc.m.functions:
        for blk in f.blocks:
            new_list = []
            for inst in blk.instructions:
                si = getattr(inst, "sync_info", None)
                waits = list(si.on_wait) if si is not None else []
                if len(waits) > MAX_ENGINE_WAITS:
                    extra = waits[: -MAX_ENGINE_WAITS]
                    keep = waits[-MAX_ENGINE_WAITS:]
                    for i in range(0, len(extra), MAX_ENGINE_WAITS):
                        nop = mybir.InstNoOp(
                            name=f"I-wsplit-{nc.next_id()}", ins=[], outs=[]
                        )
                        nop.engine = inst.engine
                        nop.sync_info = mybir.SyncInfo(
                            on_wait=extra[i : i + MAX_ENGINE_WAITS], on_update=[]
                        )
                        new_list.append(nop)
                    inst.sync_info = mybir.SyncInfo(
                        on_wait=keep, on_update=list(si.on_update)
                    )
                    n_split += 1
                new_list.append(inst)
            blk.instructions = new_list
    return n_split
# ---------------------------------------------------------------------------
# Optional: enable walrus LDWEIGHTS optimization (elide redundant weight
# loads; overlap with matmuls). Toggle with BASS_LDW_OPT=0.
import os as _os
from concourse import bass_utils as _bu

_orig_bvo = _bu.bir_verify_and_optimise


def _bvo_ldwopt(*a, **k):
    if _os.environ.get("BASS_LDW_OPT", "0") == "0":
        return _orig_bvo(*a, **k)
    orig_run = _bu.run_command

    def run2(cmd, **kw):
        cmd = [
            "--enable-ldw-opt=true" if c == "--enable-ldw-opt=false" else c
            for c in cmd
        ]
        return orig_run(cmd, **kw)

    _bu.run_command = run2
    try:
        return _orig_bvo(*a, **k)
    finally:
        _bu.run_command = orig_run


_bu.bir_verify_and_optimise = _bvo_ldwopt
# ---------------------------------------------------------------------------


def build_nc() -> bass.Bass:
    nc = bass.Bass()

    x_t = nc.declare_dram_parameter("x_t", [M, S], BF16, isOutput=False)
    w_q = nc.declare_dram_parameter("w_q", [128, MK * NP * 128], BF16, isOutput=False)
    w_k = nc.declare_dram_parameter("w_k", [128, MK * NP * 128], BF16, isOutput=False)
    w_v = nc.declare_dram_parameter("w_v", [128, MK * 512], BF16, isOutput=False)
    w_o = nc.declare_dram_parameter("w_o", [128, NP * MK * 128], BF16, isOutput=False)
    b_q = nc.declare_dram_parameter("b_q", [NP, 128], F32, isOutput=False)
    b_k = nc.declare_dram_parameter("b_k", [NP, 128], F32, isOutput=False)
    b_v = nc.declare_dram_parameter("b_v", [1, H8 * DH], BF16, isOutput=False)
    out_t = nc.declare_dram_parameter("out_t", [M, S], BF16, isOutput=True)
    den_d = nc.dram_tensor("den_d", [H8, S], F32)

    with tile.TileContext(nc) as tc:
        with (
            tc.tile_pool(name="persist", bufs=1) as persist,
            tc.tile_pool(name="wstream", bufs=1) as wpool,
        ):
            # --- resident tiles -------------------------------------------
            xt = [persist.tile([128, S], BF16, tag=f"xt{k}", name=f"xt{k}") for k in range(MK)]
            qt = [persist.tile([128, S], BF16, tag=f"qt{p}", name=f"qt{p}") for p in range(NP)]
            kt = [persist.tile([128, S], BF16, tag=f"kt{p}", name=f"kt{p}") for p in range(NP)]
            vones = [
                persist.tile([128, H8 * 65], BF16, tag=f"vones{sb}", name=f"vones{sb}")
                for sb in range(NKB)
            ]
            zt = [persist.tile([128, S], BF16, tag=f"zt{p}", name=f"zt{p}") for p in range(NP)]

            wq_all = wpool.tile([128, MK * NP * 128], BF16, tag="wq_all")
            wk_all = wpool.tile([128, MK * NP * 128], BF16, tag="wk_all")
            wv_all = wpool.tile([128, MK * 512], BF16, tag="wv_all")
            wo_all = wpool.tile([128, NP * MK * 128], BF16, tag="wo_all")
            wq = [
                [wq_all[:, (k * NP + p) * 128 : (k * NP + p + 1) * 128] for p in range(NP)]
                for k in range(MK)
            ]
            wk = [
                [wk_all[:, (k * NP + p) * 128 : (k * NP + p + 1) * 128] for p in range(NP)]
                for k in range(MK)
            ]
            wv = [wv_all[:, k * 512 : (k + 1) * 512] for k in range(MK)]
            wo = [
                [wo_all[:, (c * MK + k) * 128 : (c * MK + k + 1) * 128] for k in range(MK)]
                for c in range(NP)
            ]
            bq_t = persist.tile([128, NP], F32, tag="bq")
            bk_t = persist.tile([128, NP], F32, tag="bk")
            bv_t = persist.tile([1, H8 * DH], BF16, tag="bv")
            ones_col = persist.tile([1, 128], BF16, tag="ones_col")

            # --- loads: xt k-major on SP ring; V/O weights + biases on the
            # ACT HWDGE ring (second physical ring).
            HALF = MK * NP * 128 // 2
            nc.sync.dma_start(xt[0][:], x_t[0:128, :])
            nc.sync.dma_start(wq_all[:, 0:HALF], w_q[:, 0:HALF])
            nc.sync.dma_start(wk_all[:, 0:HALF], w_k[:, 0:HALF])
            nc.sync.dma_start(xt[1][:], x_t[128:256, :])
            nc.sync.dma_start(wq_all[:, HALF:], w_q[:, HALF:])
            nc.sync.dma_start(wk_all[:, HALF:], w_k[:, HALF:])
            for k in range(2, MK):
                nc.sync.dma_start(xt[k][:], x_t[k * 128 : (k + 1) * 128, :])
            nc.scalar.dma_start(wv_all[:], w_v[:])
            for p in range(NP):
                nc.scalar.dma_start(bq_t[:, p], b_q[p])
                nc.scalar.dma_start(bk_t[:, p], b_k[p])
            nc.scalar.dma_start(bv_t[:], b_v[:])
            nc.scalar.dma_start(wo_all[:], w_o[:])
            nc.gpsimd.memset(ones_col[:], 1.0)
            for sb in range(NKB):
                v3 = vones[sb][:].rearrange("p (h e) -> p h e", e=65)
                nc.gpsimd.memset(v3[:, :, 64:65], 1.0)

            # --- fused QKV + attention ------------------------------------
            with (
                tc.tile_pool(name="qkv_ps", bufs=2, space="PSUM") as qkv_ps,
                tc.tile_pool(name="s_ps", bufs=2, space="PSUM") as s_ps,
                tc.tile_pool(name="z_ps", bufs=2, space="PSUM") as z_ps,
                tc.tile_pool(name="epool", bufs=10) as epool,
                tc.tile_pool(name="npool", bufs=6) as npool,
            ):
                def v_proj(kb):
                    ksl = slice(kb * KB, (kb + 1) * KB)
                    ps_v = qkv_ps.tile([128, H8 * DH], F32, tag="ps_qkv", name="ps_v")
                    for k in range(MK):
                        nc.tensor.matmul(
                            ps_v[:], xt[k][:, ksl], wv[k], start=(k == 0), stop=False
                        )
                    nc.tensor.matmul(
                        ps_v[:], ones_col[:], bv_t[:], start=False, stop=True
                    )
                    v3 = vones[kb][:].rearrange("p (h e) -> p h e", e=65)
                    nc.vector.tensor_copy(
                        v3[:, :, 0:64], ps_v[:].rearrange("p (h e) -> p h e", e=64)
                    )

                def qk_proj(p, sb):
                    ssl = slice(sb * SB, (sb + 1) * SB)
                    for w_t, b_t, dst in ((wq, bq_t, qt), (wk, bk_t, kt)):
                        ps = qkv_ps.tile([128, SB], F32, tag="ps_qkv", name="ps_qk")
                        for k in range(MK):
                            nc.tensor.matmul(
                                ps[:],
                                w_t[k][p],
                                xt[k][:, ssl],
                                start=(k == 0),
                                stop=(k == MK - 1),
                            )
                        nc.vector.tensor_scalar_add(
                            dst[p][:, ssl], ps[:], b_t[:, p : p + 1]
                        )

                for kb in range(NKB):
                    v_proj(kb)
                for p in range(NP):
                    for sb in range(NSB):
                        qk_proj(p, sb)

                    # attention for this pair
                    for j in range(NSB):
                        qsl = slice(j * SB, (j + 1) * SB)
                        nk = 4 * (j + 1)
                        zps = [
                            z_ps.tile([65, SB], F32, tag="zps", name="zps0", bufs=2),
                            z_ps.tile([65, SB], F32, tag="zps", name="zps1", bufs=2),
                        ]

                        def z_mms(kbz, e_tile):
                            # crossing blocks: queries < 128i are fully masked
                            i = kbz - 4 * j
                            c0 = 128 * i if i > 0 else 0
                            for h2 in range(2):
                                h = 2 * p + h2
                                nc.tensor.matmul(
                                    zps[h2][:, c0:],
                                    vones[kbz][:, 65 * h : 65 * h + 65],
                                    e_tile[:, h2 * SB + c0 : (h2 + 1) * SB],
                                    start=(kbz == 0),
                                    stop=(kbz == nk - 1),
                                )

                        prev = None
                        for kb in range(nk):
                            ksl = slice(kb * KB, (kb + 1) * KB)
                            i = kb - 4 * j
                            c0 = 128 * i if i > 0 else 0
                            sps = s_ps.tile([128, 2 * SB], F32, tag="sps", name="sps")
                            for h2 in range(2):
                                rows = slice(h2 * 64, h2 * 64 + 64)
                                nc.tensor.matmul(
                                    sps[:, h2 * SB + c0 : (h2 + 1) * SB],
                                    kt[p][rows, ksl],
                                    qt[p][rows, j * SB + c0 : (j + 1) * SB],
                                )
                            e = epool.tile([128, 2 * SB], BF16, tag="e", name="e")
                            if c0:
                                e3 = e[:].rearrange("p (h q) -> p h q", q=SB)
                                s3 = sps[:].rearrange("p (h q) -> p h q", q=SB)
                                nc.gpsimd.memset(e3[:, :, 0:c0], 0.0)
                                nc.scalar.activation(
                                    e3[:, :, c0:],
                                    s3[:, :, c0:],
                                    mybir.ActivationFunctionType.Exp,
                                    scale=float(ATTN_SCALE),
                                )
                            else:
                                nc.scalar.activation(
                                    e[:],
                                    sps[:],
                                    mybir.ActivationFunctionType.Exp,
                                    scale=float(ATTN_SCALE),
                                )
                            if i >= 0:
                                # zero the strictly-upper part of the diagonal
                                # 128-wide stripe
                                e3 = e[:].rearrange("p (h q) -> p h q", q=SB)
                                nc.gpsimd.affine_select(
                                    out=e3[:, :, c0 : c0 + 128],
                                    in_=e3[:, :, c0 : c0 + 128],
                                    compare_op=mybir.AluOpType.is_ge,
                                    fill=0.0,
                                    base=j * SB - kb * KB + c0,
                                    pattern=[[0, 2], [1, 128]],
                                    channel_multiplier=-1,
                                )
                            if prev is not None:
                                z_mms(prev[0], prev[1])
                            prev = (kb, e)
                        z_mms(prev[0], prev[1])

                        for h2 in range(2):
                            rows = slice(h2 * 64, h2 * 64 + 64)
                            dr = npool.tile([1, SB], F32, tag="dr", name="dr")
                            nc.vector.tensor_copy(dr[:], zps[h2][64:65, :])
                            nc.sync.dma_start(den_d[2 * p + h2, qsl], dr[:])
                            nc.vector.tensor_copy(
                                zt[p][rows, qsl], zps[h2][0:64, :]
                            )


            # --- normalization + output projection ------------------------
            with (
                tc.tile_pool(name="n2pool", bufs=4) as n2pool,
                tc.tile_pool(name="bc_ps", bufs=2, space="PSUM") as bcpool,
                tc.tile_pool(name="o_ps", bufs=4, space="PSUM") as o_ps,
                tc.tile_pool(name="opool", bufs=4) as opool,
            ):
                for p in range(NP):
                    for h2 in range(2):
                        rows = slice(h2 * 64, h2 * 64 + 64)
                        den_sb = n2pool.tile([1, S], F32, tag="den_sb", name="den_sb")
                        nc.sync.dma_start(den_sb[:], den_d[2 * p + h2])
                        rcp_sb = n2pool.tile([1, S], BF16, tag="rcp_sb", name="rcp_sb")
                        act_reciprocal(nc, rcp_sb[:], den_sb[:])
                        for j in range(NSB):
                            qsl = slice(j * SB, (j + 1) * SB)
                            bc = bcpool.tile([64, SB], F32, tag="bc", name="bc")
                            nc.tensor.matmul(
                                bc[:], ones_col[:, 0:64], rcp_sb[:, qsl]
                            )
                            nc.vector.tensor_tensor(
                                zt[p][rows, qsl],
                                zt[p][rows, qsl],
                                bc[:],
                                op=mybir.AluOpType.mult,
                            )
                for k in range(MK):
                    for j in range(NSB):
                        qsl = slice(j * SB, (j + 1) * SB)
                        ps_o = o_ps.tile([128, SB], F32, tag="ps_o", name="ps_o")
                        for c in range(NP):
                            nc.tensor.matmul(
                                ps_o[:],
                                wo[c][k],
                                zt[c][:, qsl],
                                start=(c == 0),
                                stop=(c == NP - 1),
                            )
                        ot = opool.tile([128, SB], BF16, tag="ot", name="ot")
                        nc.vector.tensor_copy(ot[:], ps_o[:])
                        nc.scalar.dma_start(
                            out_t[k * 128 : (k + 1) * 128, qsl], ot[:]
                        )

    if _os.environ.get("BASS_FUSE_LDW", "0") != "0":
        refuse_ldweights(nc)
    if _os.environ.get("BASS_STRIP_TP", "0") != "0":
        strip_tile_positions(nc)
    split_multi_waits(nc)
    return nc


_CACHED = {}


def _get_nc():
    if "nc" not in _CACHED:
        _CACHED["nc"] = build_nc()
    return _CACHED["nc"]


def kernel(
    x,
    pos_embed,
    W_Q,
    b_Q,
    W_K,
    b_K,
    W_V,
    b_V,
    W_O,
    b_O,
    _want_results=False,
    _trace=False,
):
    from concourse.bass_utils import run_bass_kernel_spmd

    bf16 = ml_dtypes.bfloat16
    x = np.asarray(x, np.float32)
    W_Q = np.asarray(W_Q, np.float32)
    b_Q = np.asarray(b_Q, np.float32)
    W_K = np.asarray(W_K, np.float32)
    b_K = np.asarray(b_K, np.float32)
    W_V = np.asarray(W_V, np.float32)
    b_V = np.asarray(b_V, np.float32)
    W_O = np.asarray(W_O, np.float32)
    b_O = np.asarray(b_O, np.float32)

    in_maps = []
    for c in range(8):
        b, g = divmod(c, 2)
        hs = slice(g * H8, (g + 1) * H8)
        # [H8, M, DH] -> [M, H8*DH] with col = 64*h + d (pair-major for Q/K)
        wq = np.ascontiguousarray(W_Q[hs].transpose(1, 0, 2).reshape(M, H8 * DH))
        wk = np.ascontiguousarray(W_K[hs].transpose(1, 0, 2).reshape(M, H8 * DH))
        wv = np.ascontiguousarray(W_V[hs].transpose(1, 0, 2).reshape(M, H8 * DH))
        wo = np.ascontiguousarray(W_O[hs].reshape(H8 * DH, M))
        in_maps.append(
            {
                "x_t": np.ascontiguousarray(x[b].T).astype(bf16),
                "w_q": wq.astype(bf16),
                "w_k": wk.astype(bf16),
                "w_v": wv.astype(bf16),
                "w_o": wo.astype(bf16),
                "b_q": np.ascontiguousarray(b_Q[hs].reshape(NP, 128)),
                "b_k": np.ascontiguousarray(b_K[hs].reshape(NP, 128)),
                "b_v": b_V[hs].reshape(1, H8 * DH).astype(bf16),
            }
        )

    nc = _get_nc()
    res = run_bass_kernel_spmd(nc, in_maps, list(range(8)), trace=_trace)

    out = np.empty((B, S, M), np.float32)
    for b in range(B):
        p0 = res.results[2 * b]["out_t"].astype(np.float32)
        p1 = res.results[2 * b + 1]["out_t"].astype(np.float32)
        out[b] = (p0 + p1).T + b_O
    if _want_results:
        return out, res
    return out


# revision 25
# speedup vs baseline: 1.7763x; 1.0097x over previous
"""Causal multi-head attention block, sharded over 8 TRN2 NeuronCores.

Sharding: core c handles batch b = c//2 and head-group g = c%2 (8 of 16 heads).
Each core computes QKV projections, causal flash-style attention, and a
partial output projection for its head group; the host sums the two
head-group partials per batch (partial-sum unshard) and adds b_O.

On-device layouts (per core, S=2048, M=1024, H8=8 heads, Dh=64):
  x_t     [1024, 2048]  x[b] transposed (host pre-transposes)     bf16
  QT/KT   4 pair-tiles [128, 2048]: partition = (head-in-pair, d) bf16
  Vones   16 s-tiles [128, 520]: row=key pos, col=65*h+d, d=64 → 1.0 bf16
  zT_all  4 pair-tiles [128, 2048] (normalized z^T)               bf16
  out_t   [1024, 2048]  partial (out proj)^T, host sums + transposes
Scores are computed transposed (S^T[key, query]) so softmax denominators
come from an extra all-ones column in V (matmul partition reduction), and
no on-chip transposes are needed anywhere.
"""

import sys

if "/opt/trn_rl_repo" not in sys.path:
    sys.path.insert(0, "/opt/trn_rl_repo")

import numpy as np
import ml_dtypes

import concourse.bass as bass
import concourse.mybir as mybir
from concourse import tile

BF16 = mybir.dt.bfloat16
F32 = mybir.dt.float32

B, S, M, H, DH = 4, 2048, 1024, 16, 64
H8 = 8          # heads per core
NP = 4          # head pairs per core
SB = 512        # query superblock
KB = 128        # key block
NSB = S // SB   # 4
NKB = S // KB   # 16
MK = M // 128   # 8 contraction chunks
ATTN_SCALE = 1.0 / np.sqrt(DH)

# ---------------------------------------------------------------------------
# Patch: this walrus build rejects >1 sync-wait per engine instruction.
# Post-pass: for any non-DMA instruction with N>1 waits, insert N-1
# single-wait NoOps on the same engine immediately before it.
MAX_ENGINE_WAITS = 1


def strip_tile_positions(nc: bass.Bass):
    # walrus's LDW optimization refuses instructions carrying tile_position;
    # bass auto-derives it from AP base partitions, which walrus can also do.
    n = 0
    for f in nc.m.functions:
        for blk in f.blocks:
            for inst in blk.instructions:
                if isinstance(inst, (mybir.InstLdweights, mybir.InstMatmult)):
                    if getattr(inst, "tile_position", None) is not None:
                        inst.tile_position = None
                        n += 1
                    if getattr(inst, "tile_size", None) is not None:
                        inst.tile_size = None
    return n


def refuse_ldweights(nc: bass.Bass):
    # Tile's lowering splits each matmul into standalone Ldweights + Matmult
    # (ldweights=False). Re-fuse: drop the Ldweights, mark the matmult
    # self-loading, and merge the Ldweights' semaphore waits into it
    # (split_multi_waits runs after and redistributes >1-wait cases).
    n = 0
    for f in nc.m.functions:
        for blk in f.blocks:
            new_list = []
            pend_waits = []
            pend_updates = []
            for inst in blk.instructions:
                if isinstance(inst, mybir.InstLdweights):
                    si = inst.sync_info
                    if si is not None:
                        pend_waits.extend(list(si.on_wait))
                        pend_updates.extend(list(si.on_update))
                    n += 1
                    continue
                if isinstance(inst, mybir.InstMatmult) and (
                    pend_waits or pend_updates
                ):
                    inst.ldweights = True
                    si = inst.sync_info
                    w = list(si.on_wait) if si else []
                    u = list(si.on_update) if si else []
                    inst.sync_info = mybir.SyncInfo(
                        on_wait=pend_waits + w, on_update=u + pend_updates
                    )
                    pend_waits, pend_updates = [], []
                elif isinstance(inst, mybir.InstMatmult):
                    inst.ldweights = True
                new_list.append(inst)
            assert not pend_waits and not pend_updates
            blk.instructions = new_list
    return n


def split_multi_waits(nc: bass.Bass):
    n_split = 0
    for f in nc.m.functions:
        for blk in f.blocks:
            new_list = []
            for inst in blk.instructions:
                si = getattr(inst, "sync_info", None)
                waits = list(si.on_wait) if si is not None else []
                if len(waits) > MAX_ENGINE_WAITS:
                    extra = waits[: -MAX_ENGINE_WAITS]
                    keep = waits[-MAX_ENGINE_WAITS:]
                    for i in range(0, len(extra), MAX_ENGINE_WAITS):
                        nop = mybir.InstNoOp(
                            name=f"I-wsplit-{nc.next_id()}", ins=[], outs=[]
                        )
                        nop.engine = inst.engine
                        nop.sync_info = mybir.SyncInfo(
                            on_wait=extra[i : i + MAX_ENGINE_WAITS], on_update=[]
                        )
                        new_list.append(nop)
                    inst.sync_info = mybir.SyncInfo(
                        on_wait=keep, on_update=list(si.on_update)
                    )
                    n_split += 1
                new_list.append(inst)
            blk.instructions = new_list
    return n_split
# ---------------------------------------------------------------------------
# Optional: enable walrus LDWEIGHTS optimization (elide redundant weight
# loads; overlap with matmuls). Toggle with BASS_LDW_OPT=0.
import os as _os
from concourse import bass_utils as _bu

_orig_bvo = _bu.bir_verify_and_optimise


def _bvo_ldwopt(*a, **k):
    if _os.environ.get("BASS_LDW_OPT", "0") == "0":
        return _orig_bvo(*a, **k)
    orig_run = _bu.run_command

    def run2(cmd, **kw):
        cmd = [
            "--enable-ldw-opt=true" if c == "--enable-ldw-opt=false" else c
            for c in cmd
        ]
        return orig_run(cmd, **kw)

    _bu.run_command = run2
    try:
        return _orig_bvo(*a, **k)
    finally:
        _bu.run_command = orig_run


_bu.bir_verify_and_optimise = _bvo_ldwopt
# ---------------------------------------------------------------------------


def build_nc() -> bass.Bass:
    nc = bass.Bass()

    x_t = nc.declare_dram_parameter("x_t", [M, S], BF16, isOutput=False)
    w_q = nc.declare_dram_parameter("w_q", [128, MK * NP * 128], BF16, isOutput=False)
    w_k = nc.declare_dram_parameter("w_k", [128, MK * NP * 128], BF16, isOutput=False)
    w_v = nc.declare_dram_parameter("w_v", [128, MK * 512], BF16, isOutput=False)
    w_o = nc.declare_dram_parameter("w_o", [128, NP * MK * 128], BF16, isOutput=False)
    b_q = nc.declare_dram_parameter("b_q", [NP, 128], F32, isOutput=False)
    b_k = nc.declare_dram_parameter("b_k", [NP, 128], F32, isOutput=False)
    b_v = nc.declare_dram_parameter("b_v", [1, H8 * DH], BF16, isOutput=False)
    out_t = nc.declare_dram_parameter("out_t", [M, S], BF16, isOutput=True)
    den_d = nc.dram_tensor("den_d", [H8, S], F32)

    with tile.TileContext(nc) as tc:
        with (
            tc.tile_pool(name="persist", bufs=1) as persist,
            tc.tile_pool(name="wstream", bufs=1) as wpool,
        ):
            # --- resident tiles -------------------------------------------
            xt = [persist.tile([128, S], BF16, tag=f"xt{k}", name=f"xt{k}") for k in range(MK)]
            qt = [persist.tile([128, S], BF16, tag=f"qt{p}", name=f"qt{p}") for p in range(NP)]
            kt = [persist.tile([128, S], BF16, tag=f"kt{p}", name=f"kt{p}") for p in range(NP)]
            vones = [
                persist.tile([128, H8 * 65], BF16, tag=f"vones{sb}", name=f"vones{sb}")
                for sb in range(NKB)
            ]
            zt = [persist.tile([128, S], BF16, tag=f"zt{p}", name=f"zt{p}") for p in range(NP)]

            wq_all = wpool.tile([128, MK * NP * 128], BF16, tag="wq_all")
            wk_all = wpool.tile([128, MK * NP * 128], BF16, tag="wk_all")
            wv_all = wpool.tile([128, MK * 512], BF16, tag="wv_all")
            wo_all = wpool.tile([128, NP * MK * 128], BF16, tag="wo_all")
            wq = [
                [wq_all[:, (k * NP + p) * 128 : (k * NP + p + 1) * 128] for p in range(NP)]
                for k in range(MK)
            ]
            wk = [
                [wk_all[:, (k * NP + p) * 128 : (k * NP + p + 1) * 128] for p in range(NP)]
                for k in range(MK)
            ]
            wv = [wv_all[:, k * 512 : (k + 1) * 512] for k in range(MK)]
            wo = [
                [wo_all[:, (c * MK + k) * 128 : (c * MK + k + 1) * 128] for k in range(MK)]
                for c in range(NP)
            ]
            bq_t = persist.tile([128, NP], F32, tag="bq")
            bk_t = persist.tile([128, NP], F32, tag="bk")
            bv_t = persist.tile([1, H8 * DH], BF16, tag="bv")
            ones_col = persist.tile([1, 128], BF16, tag="ones_col")

            # --- loads: xt k-major on SP ring; V/O weights + biases on the
            # ACT HWDGE ring (second physical ring).
            HALF = MK * NP * 128 // 2
            nc.sync.dma_start(xt[0][:], x_t[0:128, :])
            nc.sync.dma_start(wq_all[:, 0:HALF], w_q[:, 0:HALF])
            nc.sync.dma_start(wk_all[:, 0:HALF], w_k[:, 0:HALF])
            nc.sync.dma_start(xt[1][:], x_t[128:256, :])
            nc.sync.dma_start(wq_all[:, HALF:], w_q[:, HALF:])
            nc.sync.dma_start(wk_all[:, HALF:], w_k[:, HALF:])
            for k in range(2, MK):
                nc.sync.dma_start(xt[k][:], x_t[k * 128 : (k + 1) * 128, :])
            nc.scalar.dma_start(wv_all[:], w_v[:])
            for p in range(NP):
                nc.scalar.dma_start(bq_t[:, p], b_q[p])
                nc.scalar.dma_start(bk_t[:, p], b_k[p])
            nc.scalar.dma_start(bv_t[:], b_v[:])
            nc.scalar.dma_start(wo_all[:], w_o[:])
            nc.gpsimd.memset(ones_col[:], 1.0)
            for sb in range(NKB):
                v3 = vones[sb][:].rearrange("p (h e) -> p h e", e=65)
                nc.gpsimd.memset(v3[:, :, 64:65], 1.0)

            # --- fused QKV + attention ------------------------------------
            with (
                tc.tile_pool(name="qkv_ps", bufs=2, space="PSUM") as qkv_ps,
                tc.tile_pool(name="s_ps", bufs=2, space="PSUM") as s_ps,
                tc.tile_pool(name="z_ps", bufs=2, space="PSUM") as z_ps,
                tc.tile_pool(name="epool", bufs=10) as epool,
                tc.tile_pool(name="npool", bufs=6) as npool,
            ):
                def v_proj(kb):
                    ksl = slice(kb * KB, (kb + 1) * KB)
                    ps_v = qkv_ps.tile([128, H8 * DH], F32, tag="ps_qkv", name="ps_v")
                    for k in range(MK):
                        nc.tensor.matmul(
                            ps_v[:], xt[k][:, ksl], wv[k], start=(k == 0), stop=False
                        )
                    nc.tensor.matmul(
                        ps_v[:], ones_col[:], bv_t[:], start=False, stop=True
                    )
                    v3 = vones[kb][:].rearrange("p (h e) -> p h e", e=65)
                    nc.vector.tensor_copy(
                        v3[:, :, 0:64], ps_v[:].rearrange("p (h e) -> p h e", e=64)
                    )

                def qk_proj(p, sb):
                    ssl = slice(sb * SB, (sb + 1) * SB)
                    for w_t, b_t, dst in ((wq, bq_t, qt), (wk, bk_t, kt)):
                        ps = qkv_ps.tile([128, SB], F32, tag="ps_qkv", name="ps_qk")
                        for k in range(MK):
                            nc.tensor.matmul(
                                ps[:],
                                w_t[k][p],
                                xt[k][:, ssl],
                                start=(k == 0),
                                stop=(k == MK - 1),
                            )
                        nc.vector.tensor_scalar_add(
                            dst[p][:, ssl], ps[:], b_t[:, p : p + 1]
                        )

                for kb in range(NKB):
                    v_proj(kb)
                for p in range(NP):
                    for sb in range(NSB):
                        qk_proj(p, sb)

                    # attention for this pair
                    for j in range(NSB):
                        qsl = slice(j * SB, (j + 1) * SB)
                        nk = 4 * (j + 1)
                        zps = [
                            z_ps.tile([65, SB], F32, tag="zps", name="zps0", bufs=2),
                            z_ps.tile([65, SB], F32, tag="zps", name="zps1", bufs=2),
                        ]

                        def z_mms(kbz, e_tile):
                            # crossing blocks: queries < 128i are fully masked
                            i = kbz - 4 * j
                            c0 = 128 * i if i > 0 else 0
                            for h2 in range(2):
                                h = 2 * p + h2
                                nc.tensor.matmul(
                                    zps[h2][:, c0:],
                                    vones[kbz][:, 65 * h : 65 * h + 65],
                                    e_tile[:, h2 * SB + c0 : (h2 + 1) * SB],
                                    start=(kbz == 0),
                                    stop=(kbz == nk - 1),
                                )

                        pend = []
                        for kb in range(nk):
                            ksl = slice(kb * KB, (kb + 1) * KB)
                            i = kb - 4 * j
                            c0 = 128 * i if i > 0 else 0
                            sps = s_ps.tile([128, 2 * SB], F32, tag="sps", name="sps")
                            for h2 in range(2):
                                rows = slice(h2 * 64, h2 * 64 + 64)
                                nc.tensor.matmul(
                                    sps[:, h2 * SB + c0 : (h2 + 1) * SB],
                                    kt[p][rows, ksl],
                                    qt[p][rows, j * SB + c0 : (j + 1) * SB],
                                )
                            e = epool.tile([128, 2 * SB], BF16, tag="e", name="e")
                            if c0:
                                e3 = e[:].rearrange("p (h q) -> p h q", q=SB)
                                s3 = sps[:].rearrange("p (h q) -> p h q", q=SB)
                                nc.gpsimd.memset(e3[:, :, 0:c0], 0.0)
                                nc.scalar.activation(
                                    e3[:, :, c0:],
                                    s3[:, :, c0:],
                                    mybir.ActivationFunctionType.Exp,
                                    scale=float(ATTN_SCALE),
                                )
                            else:
                                nc.scalar.activation(
                                    e[:],
                                    sps[:],
                                    mybir.ActivationFunctionType.Exp,
                                    scale=float(ATTN_SCALE),
                                )
                            if i >= 0:
                                # zero the strictly-upper part of the diagonal
                                # 128-wide stripe
                                e3 = e[:].rearrange("p (h q) -> p h q", q=SB)
                                nc.gpsimd.affine_select(
                                    out=e3[:, :, c0 : c0 + 128],
                                    in_=e3[:, :, c0 : c0 + 128],
                                    compare_op=mybir.AluOpType.is_ge,
                                    fill=0.0,
                                    base=j * SB - kb * KB + c0,
                                    pattern=[[0, 2], [1, 128]],
                                    channel_multiplier=-1,
                                )
                            pend.append((kb, e))
                            if len(pend) > 2:
                                z_mms(*pend.pop(0))
                        for it in pend:
                            z_mms(*it)

                        for h2 in range(2):
                            rows = slice(h2 * 64, h2 * 64 + 64)
                            dr = npool.tile([1, SB], F32, tag="dr", name="dr")
                            nc.vector.tensor_copy(dr[:], zps[h2][64:65, :])
                            nc.sync.dma_start(den_d[2 * p + h2, qsl], dr[:])
                            nc.vector.tensor_copy(
                                zt[p][rows, qsl], zps[h2][0:64, :]
                            )


            # --- normalization + output projection ------------------------
            with (
                tc.tile_pool(name="n2pool", bufs=4) as n2pool,
                tc.tile_pool(name="bc_ps", bufs=2, space="PSUM") as bcpool,
                tc.tile_pool(name="o_ps", bufs=4, space="PSUM") as o_ps,
                tc.tile_pool(name="opool", bufs=4) as opool,
            ):
                for p in range(NP):
                    for h2 in range(2):
                        rows = slice(h2 * 64, h2 * 64 + 64)
                        den_sb = n2pool.tile([1, S], F32, tag="den_sb", name="den_sb")
                        nc.sync.dma_start(den_sb[:], den_d[2 * p + h2])
                        rcp_sb = n2pool.tile([1, S], BF16, tag="rcp_sb", name="rcp_sb")
                        act_reciprocal(nc, rcp_sb[:], den_sb[:])
                        for j in range(NSB):
                            qsl = slice(j * SB, (j + 1) * SB)
                            bc = bcpool.tile([64, SB], F32, tag="bc", name="bc")
                            nc.tensor.matmul(
                                bc[:], ones_col[:, 0:64], rcp_sb[:, qsl]
                            )
                            nc.vector.tensor_tensor(
                                zt[p][rows, qsl],
                                zt[p][rows, qsl],
                                bc[:],
                                op=mybir.AluOpType.mult,
                            )
                for k in range(MK):
                    for j in range(NSB):
                        qsl = slice(j * SB, (j + 1) * SB)
                        ps_o = o_ps.tile([128, SB], F32, tag="ps_o", name="ps_o")
                        for c in range(NP):
                            nc.tensor.matmul(
                                ps_o[:],
                                wo[c][k],
                                zt[c][:, qsl],
                                start=(c == 0),
                                stop=(c == NP - 1),
                            )
                        ot = opool.tile([128, SB], BF16, tag="ot", name="ot")
                        nc.vector.tensor_copy(ot[:], ps_o[:])
                        nc.scalar.dma_start(
                            out_t[k * 128 : (k + 1) * 128, qsl], ot[:]
                        )

    if _os.environ.get("BASS_FUSE_LDW", "0") != "0":
        refuse_ldweights(nc)
    if _os.environ.get("BASS_STRIP_TP", "0") != "0":
        strip_tile_positions(nc)
    split_multi_waits(nc)
    return nc


_CACHED = {}


def _get_nc():
    if "nc" not in _CACHED:
        _CACHED["nc"] = build_nc()
    return _CACHED["nc"]


def kernel(
    x,
    pos_embed,
    W_Q,
    b_Q,
    W_K,
    b_K,
    W_V,
    b_V,
    W_O,
    b_O,
    _want_results=False,
    _trace=False,
):
    from concourse.bass_utils import run_bass_kernel_spmd

    bf16 = ml_dtypes.bfloat16
    x = np.asarray(x, np.float32)
    W_Q = np.asarray(W_Q, np.float32)
    b_Q = np.asarray(b_Q, np.float32)
    W_K = np.asarray(W_K, np.float32)
    b_K = np.asarray(b_K, np.float32)
    W_V = np.asarray(W_V, np.float32)
    b_V = np.asarray(b_V, np.float32)
    W_O = np.asarray(W_O, np.float32)
    b_O = np.asarray(b_O, np.float32)

    in_maps = []
    for c in range(8):
        b, g = divmod(c, 2)
        hs = slice(g * H8, (g + 1) * H8)
        # [H8, M, DH] -> [M, H8*DH] with col = 64*h + d (pair-major for Q/K)
        wq = np.ascontiguousarray(W_Q[hs].transpose(1, 0, 2).reshape(M, H8 * DH))
        wk = np.ascontiguousarray(W_K[hs].transpose(1, 0, 2).reshape(M, H8 * DH))
        wv = np.ascontiguousarray(W_V[hs].transpose(1, 0, 2).reshape(M, H8 * DH))
        wo = np.ascontiguousarray(W_O[hs].reshape(H8 * DH, M))
        in_maps.append(
            {
                "x_t": np.ascontiguousarray(x[b].T).astype(bf16),
                "w_q": wq.astype(bf16),
                "w_k": wk.astype(bf16),
                "w_v": wv.astype(bf16),
                "w_o": wo.astype(bf16),
                "b_q": np.ascontiguousarray(b_Q[hs].reshape(NP, 128)),
                "b_k": np.ascontiguousarray(b_K[hs].reshape(NP, 128)),
                "b_v": b_V[hs].reshape(1, H8 * DH).astype(bf16),
            }
        )

    nc = _get_nc()
    res = run_bass_kernel_spmd(nc, in_maps, list(range(8)), trace=_trace)

    out = np.empty((B, S, M), np.float32)
    for b in range(B):
        p0 = res.results[2 * b]["out_t"].astype(np.float32)
        p1 = res.results[2 * b + 1]["out_t"].astype(np.float32)
        out[b] = (p0 + p1).T + b_O
    if _want_results:
        return out, res
    return out
